# revision 4
# baseline (speedup 1.0000x reference)
"""Bass/Trainium2 kernel for the decomposed LocallyConnected2d layer.

out[b,o,i,j] = sum_{c,k} x[b, c, i+di, j+dj] * w[o, c, i, j, k] + bias[o,i,j]
with k = di*3 + dj (3x3 kernel, stride 1).

Strategy: shard over output rows i across 8 cores (4 rows each). The kernel
is HBM-read-bandwidth bound (per-core: 4.72 MB of per-location weight +
1.67 MB of x, fp16), and measured per-engine SDMA read rate is ~25% higher
for 128-partition transfers than 96-partition ones. So ALL inbound DMAs land
in 128-partition staging tiles with ~4-6.5 KB contiguous runs, and the
96-partition matmul operand layouts are built on-chip with DVE/ACT
cross-quadrant copies (nch=32 bank-0 routing: a 32-partition copy may read
quadrant a and write quadrant b), which are free relative to the DMA wall.

Contraction (c,di,dj)=288 is split into 3 chunks of 96 = (dj,c) indexed,
chunked over di; chunk di of location (i,j) is a matmul
lhsT=[96,64] rhs=[96,128] accumulating into PSUM [64 o, 128 b]. x is stored
once (deduplicated) as xdup[96, 6, 34, 128] where partition dj*32+c holds
x[c] column-shifted by dj - the three shifted copies are built on-chip
instead of re-read from HBM. Bias is NOT applied on device; the host adds it
in fp32 after the gather (cheaper than a 97th contraction row and removes
ones/bias DMAs). Even/odd j use PE column groups 0/1 (tile_position) so two
locations' matmuls overlap in the array. Output is written fp16 and upcast
on the host.

Scheduling: x staging is split across both HWDGE rings (sync+scalar) so it
lands at full aggregate rate before weights; weight staging tiles stream on
sync in (row, group) order so row-i matmuls unblock as early as possible;
out DMAs ride the scalar ring, which is idle after x. PSUM->SBUF copies run
on whichever of vector/scalar is free; requadrant copies split across
vector+scalar.
"""

import sys

for _p in ("/opt/trn_rl_repo", "/root/.axon_site/_ro/trn_rl_repo"):
    if _p not in sys.path:
        sys.path.append(_p)

import numpy as np

B = 128
C_IN = 32
C_OUT = 64
OH = OW = 32
KH = KW = 3
H = W = 34
N_CORES = 8
RPC = OH // N_CORES          # output rows per core = 4
HALO = RPC + KH - 1          # x rows per core = 6
NPAIR = OW // 2              # j-pairs per row = 16
NGRP = 4                     # j-pairs per psum group
GRPS = NPAIR // NGRP         # psum groups per row = 4
NUNIT = RPC * KH * KW        # w requadrant units = 36
NWTILE = NUNIT // 4          # 128-partition w staging tiles = 9

_DT_MM = "float16"
_DT_OUT = "float16"

_prog_cache = {}


def _build_program():
    import concourse.tile as tile
    from concourse import bacc, mybir

    dt_mm = getattr(mybir.dt, _DT_MM)
    dt_out = getattr(mybir.dt, _DT_OUT)
    f32 = mybir.dt.float32

    nc = bacc.Bacc("TRN2", target_bir_lowering=False, debug=False,
                   num_devices=N_CORES)

    # Per-core DRAM I/O (host pre-sharded / pre-transposed, fp16):
    #   x_in [128, 6, 34, 32]   partition bq*32+c holds x[c, h, w, 32bq+bl]
    #   w_in [9, 128, 32, 64]   tile t partition q*32+c holds unit u=4t+q,
    #                           u = i*9 + (di*3+dj); content w[o,c,i,j,k]
    #   out  [p2=128 (par*64+o), i=4, jh=16, b=128] ; j = 2*jh + par
    x_in = nc.dram_tensor("x", [B, HALO, W, C_IN], dt_mm,
                          kind="ExternalInput").ap()
    w_in = nc.dram_tensor("w", [NWTILE, B, OW, C_OUT], dt_mm,
                          kind="ExternalInput").ap()
    out = nc.dram_tensor("out", [B, RPC, NPAIR, B], dt_out,
                         kind="ExternalOutput").ap()

    with tile.TileContext(nc) as tc:
        with (
            tc.tile_pool(name="xstg", bufs=1) as xsp,
            tc.tile_pool(name="xdupp", bufs=1) as xdp,
            tc.tile_pool(name="wstg", bufs=5) as wsp,
            tc.tile_pool(name="wtp", bufs=1) as wtp,
            tc.tile_pool(name="opool", bufs=3) as opool,
            tc.tile_pool(name="pspool", bufs=6, space="PSUM") as pspool,
        ):
            # x staging: two h-halves, one per HWDGE ring, so x lands at the
            # full aggregate read rate before the weight stream ramps.
            xs0 = xsp.tile([B, KH, W, 32], dt_mm, tag="xs0")
            xs1 = xsp.tile([B, KH, W, 32], dt_mm, tag="xs1")
            nc.scalar.dma_start(xs0[:], x_in[:, 0:KH])
            nc.sync.dma_start(xs1[:], x_in[:, KH:HALO])

            # w staging tiles stream on sync in unit order (i-major), so
            # row 0's nine (di,dj) groups land first.
            wstg = [wsp.tile([B, OW, C_OUT], dt_mm, tag="ws",
                             name=f"ws{t}")
                    for t in range(NWTILE)]
            for t in range(NWTILE):
                nc.sync.dma_start(wstg[t][:], w_in[t])

            # xdup[dj*32+c, h, w', b] = x[c, h, w'+dj, b]; built by 24
            # cross-quadrant shift copies (3 dj x 4 bq x 2 h-halves).
            xdup = xdp.tile([96, HALO, W, B], dt_mm, tag="xdup")
            xeng = [nc.vector.tensor_copy, nc.scalar.copy]
            k = 0
            for hh, xs in ((0, xs0), (1, xs1)):
                for dj in range(KW):
                    for bq in range(4):
                        dst = xdup[32 * dj:32 * dj + 32,
                                   KH * hh:KH * hh + KH,
                                   0:W - dj, 32 * bq:32 * bq + 32]
                        src = xs[32 * bq:32 * bq + 32, :, dj:W, :]
                        xeng[k % 2](dst, src)
                        k += 1

            # w requadrant: unit u=4t+q -> wtile[i][di] quadrant dj.
            wtiles = [[wtp.tile([96, OW, C_OUT], dt_mm, tag=f"w{i}_{d}",
                                name=f"w{i}_{d}")
                       for d in range(KH)] for i in range(RPC)]
            for t in range(NWTILE):
                for q in range(4):
                    u = 4 * t + q
                    i, g = divmod(u, KH * KW)
                    di, dj = divmod(g, KW)
                    dst = wtiles[i][di][32 * dj:32 * dj + 32, :, :]
                    src = wstg[t][32 * q:32 * q + 32, :, :]
                    xeng[k % 2](dst, src)
                    k += 1

            # Matmul stream: per row, 4 PSUM groups of 4 j-pairs; chunk di
            # accumulates; even/odd j on PE column groups 0/1.
            cpeng = [nc.vector.tensor_copy, nc.scalar.copy]
            for i in range(RPC):
                out_row = opool.tile([B, NPAIR, B], dt_out, tag="op")
                for g in range(GRPS):
                    ps = pspool.tile([B, NGRP, B], f32)
                    for pig in range(NGRP):
                        for par in range(2):
                            j = 2 * (NGRP * g + pig) + par
                            pslice = ps[64 * par:64 * par + 64, pig, :]
                            tp = (0, 64 * par)
                            for di in range(KH):
                                nc.tensor.matmul(
                                    pslice,
                                    wtiles[i][di][:, j, :],
                                    xdup[0:96, i + di, j, :],
                                    start=(di == 0), stop=(di == KH - 1),
                                    tile_position=tp)
                    dst = out_row[:, NGRP * g:NGRP * (g + 1), :]
                    cpeng[g % 2](dst, ps[:])
                    if g == 1:
                        nc.scalar.dma_start(out[:, i, 0:NPAIR // 2, :],
                                            out_row[:, 0:NPAIR // 2, :])
                if GRPS > 2:
                    nc.scalar.dma_start(out[:, i, NPAIR // 2:, :],
                                        out_row[:, NPAIR // 2:, :])

    nc.compile()
    return nc


def _host_prep(x, weight):
    """Full fp32 inputs -> list of per-core input dicts."""
    np_mm = np.dtype(_DT_MM)
    # x: (B, C, H, W) -> staging [128=(bq*32+c), h, w, 32]
    x_t = np.ascontiguousarray(x.transpose(1, 2, 3, 0)).astype(np_mm)
    # w: (O, C, I, J, K) -> [i, di, dj, c, j, o] -> tiles [9, 128, 32, 64]
    w_r = weight.reshape(C_OUT, C_IN, OH, OW, KH, KW)
    w_t = w_r.transpose(2, 4, 5, 1, 3, 0).astype(np_mm)  # (i, di, dj, c, j, o)

    in_maps = []
    for m in range(N_CORES):
        r0 = m * RPC
        xc = x_t[:, r0:r0 + HALO]                         # (c, 6, 34, b)
        xs = xc.reshape(C_IN, HALO, W, 4, 32).transpose(3, 0, 1, 2, 4)
        xs = np.ascontiguousarray(xs).reshape(B, HALO, W, 32)
        wc = w_t[r0:r0 + RPC]                             # (4, 3, 3, c, j, o)
        wc = wc.reshape(NUNIT, C_IN, OW, C_OUT)
        wc = np.ascontiguousarray(wc).reshape(NWTILE, B, OW, C_OUT)
        in_maps.append({"x": xs, "w": wc})
    return in_maps


def _gather(results, bias):
    out_full = np.empty((B, C_OUT, OH, OW), np.float32)
    for m in range(N_CORES):
        r = results[m]["out"].astype(np.float32)          # (128, 4, 16, 128)
        r = r.reshape(2, C_OUT, RPC, NPAIR, B)            # par,o,i,jh,b
        r = r.transpose(4, 1, 2, 3, 0)                    # b,o,i,jh,par
        out_full[:, :, m * RPC:(m + 1) * RPC, :] = r.reshape(B, C_OUT, RPC, OW)
    out_full += bias[None].astype(np.float32)
    return out_full


def kernel(x, weight, bias, _trace=False):
    from concourse.bass_utils import run_bass_kernel_spmd

    if "nc" not in _prog_cache:
        _prog_cache["nc"] = _build_program()
    nc = _prog_cache["nc"]

    in_maps = _host_prep(np.asarray(x), np.asarray(weight))
    res = run_bass_kernel_spmd(nc, in_maps, core_ids=list(range(N_CORES)),
                               trace=_trace)
    out = _gather(res.results, np.asarray(bias))
    if _trace:
        _prog_cache["last_result"] = res
    return out


# revision 5
# speedup vs baseline: 2.2278x; 2.2278x over previous
"""Bass/Trainium2 kernel for the decomposed LocallyConnected2d layer.

out[b,o,i,j] = sum_{c,k} x[b, c, i+di, j+dj] * w[o, c, i, j, k] + bias[o,i,j]
with k = di*3 + dj (3x3 kernel, stride 1).

Strategy: shard over output rows i across 8 cores (4 rows each). Each core
owns 1/8 of the per-location weight (the dominant traffic) and a 6-row halo
slice of x. Per output location (i,j) the contraction (c,k)=288 is split into
3 chunks of 96 = (c,di) indexed, chunked over dj; each chunk is one matmul
lhsT=[96,64] rhs=[96,128] accumulating into PSUM [64 o, 128 b]. Even/odd j
use PE column groups 0/1 (tile_position) so two locations' matmuls overlap
in the array. All matmul data is fp16 (fp32 accumulate in PSUM); output is
written fp16 and upcast on the host. Bias is added on the host in fp32
(cheaper than a 97th contraction row; removes the ones/bias DMAs).

The kernel is HBM-read-bound: per-core inbound is w 4.72 MB + x 3.34 MB
(dup factor 2: partition (c,di) keeps rows i+di as one shared 4-row window)
at a measured ~230 GB/s for 96-partition reads. Scheduling is therefore
everything: ALL inbound DMAs ride ONE HWDGE ring (sync) in output-row order
(xs0, w-row0, xsr-row1, w-row1, ...) so row i's matmuls unblock after
(i+1)/4 of the stream instead of after all of it; every transfer keeps
4-8.7 KB contiguous runs per partition (large packets starve the other
queue, tiny ones pay per-packet overhead). Output DMAs (HBM writes run
~1.8x faster per engine than reads) ride the otherwise-idle scalar ring at
half-row granularity so the write tail after the last matmul is short.
PSUM->SBUF copies alternate Vector/Scalar engines.
"""

import sys

for _p in ("/opt/trn_rl_repo", "/root/.axon_site/_ro/trn_rl_repo"):
    if _p not in sys.path:
        sys.path.append(_p)

import numpy as np

B = 128
C_IN = 32
C_OUT = 64
OH = OW = 32
KH = KW = 3
H = W = 34
N_CORES = 8
RPC = OH // N_CORES          # output rows per core = 4
HALO = RPC + KH - 1          # x rows per core = 6
NPAIR = OW // 2              # j-pairs per row = 16
NGRP = 4                     # j-pairs per psum group
GRPS = NPAIR // NGRP         # psum groups per row = 4

_DT_MM = "float16"
_DT_OUT = "float16"

_prog_cache = {}


def _build_program():
    import concourse.tile as tile
    from concourse import bacc, mybir
    from bass_rust import AP

    dt_mm = getattr(mybir.dt, _DT_MM)
    dt_out = getattr(mybir.dt, _DT_OUT)
    f32 = mybir.dt.float32

    nc = bacc.Bacc("TRN2", target_bir_lowering=False, debug=False,
                   num_devices=N_CORES)

    # Per-core DRAM I/O (host pre-sharded / pre-transposed, fp16):
    #   x_in [c=32, h=6, w=34, b=128]  halo slice, b innermost
    #   w_in [r=288, i=4, j=32, o=64]  r = dj*96 + c*3 + di
    #   out  [p2=128 (par*64+o), i=4, jh=16, b=128] ; j = 2*jh + par
    x_in = nc.dram_tensor("x", [C_IN, HALO, W, B], dt_mm,
                          kind="ExternalInput").ap()
    w_in = nc.dram_tensor("w", [KW * 96, RPC, OW, C_OUT], dt_mm,
                          kind="ExternalInput").ap()
    out = nc.dram_tensor("out", [B, RPC, NPAIR, B], dt_out,
                         kind="ExternalOutput").ap()

    HSTR = W * B                # x_in h-row stride (elements)
    CSTR = HALO * W * B         # x_in c stride

    with tile.TileContext(nc) as tc:
        with (
            tc.tile_pool(name="xpool", bufs=1) as xpool,
            tc.tile_pool(name="wpool", bufs=1) as wpool,
            tc.tile_pool(name="opool", bufs=3) as opool,
            tc.tile_pool(name="pspool", bufs=6, space="PSUM") as pspool,
        ):
            # x slabs: partition p = c*3+di. xs0 serves row 0 (partition
            # holds image row di, 8.7KB runs); xsr serves rows 1..3
            # (partition holds rows 1+di..3+di, one shared 3-row window,
            # split into 3 single-row DMAs so packets stay small and row
            # i's slice lands before row i+1's weights).
            xs0 = xpool.tile([96, W, B], dt_mm, tag="xs0")
            xsr = xpool.tile([96, RPC - 1, W, B], dt_mm, tag="xsr")
            # w tiles: per (row, chunk), 4KB runs.
            wt = [[wpool.tile([96, OW, C_OUT], dt_mm, tag=f"w{i}{dj}",
                              name=f"w{i}{dj}")
                   for dj in range(KW)] for i in range(RPC)]

            def dma_w_row(i):
                for dj in range(KW):
                    nc.sync.dma_start(wt[i][dj][:],
                                      w_in[dj * 96:(dj + 1) * 96, i])

            # Single-ring row-ordered inbound stream.
            src0 = AP(x_in.tensor, 0, [(CSTR, C_IN), (HSTR, KH), (1, W * B)])
            nc.sync.dma_start(xs0[:], src0)
            dma_w_row(0)
            for s in range(RPC - 1):
                srcs = AP(x_in.tensor, HSTR * (1 + s),
                          [(CSTR, C_IN), (HSTR, KH), (1, W * B)])
                nc.sync.dma_start(xsr[:, s, :, :], srcs)
                dma_w_row(s + 1)

            def rhs(i, jj):
                if i == 0:
                    return xs0[0:96, jj, :]
                return xsr[0:96, i - 1, jj, :]

            cpeng = [nc.vector.tensor_copy, nc.scalar.copy]

            for i in range(RPC):
                out_row = opool.tile([B, NPAIR, B], dt_out, tag="op")
                for g in range(GRPS):
                    ps = pspool.tile([B, NGRP, B], f32)
                    for pig in range(NGRP):
                        for par in range(2):
                            j = 2 * (NGRP * g + pig) + par
                            pslice = ps[64 * par:64 * par + 64, pig, :]
                            tp = (0, 64 * par)
                            for dj in range(KW):
                                nc.tensor.matmul(
                                    pslice, wt[i][dj][:, j, :],
                                    rhs(i, j + dj),
                                    start=(dj == 0), stop=(dj == KW - 1),
                                    tile_position=tp)
                    dst = out_row[:, NGRP * g:NGRP * (g + 1), :]
                    cpeng[g % 2](dst, ps[:])
                    if g == 1:
                        nc.scalar.dma_start(out[:, i, 0:NPAIR // 2, :],
                                            out_row[:, 0:NPAIR // 2, :])
                nc.scalar.dma_start(out[:, i, NPAIR // 2:, :],
                                    out_row[:, NPAIR // 2:, :])

    nc.compile()
    return nc


def _host_prep(x, weight):
    """Full fp32 inputs -> list of per-core input dicts."""
    np_mm = np.dtype(_DT_MM)
    # x: (B, C, H, W) -> (C, H, W, B)
    x_t = np.ascontiguousarray(x.transpose(1, 2, 3, 0)).astype(np_mm)
    # w: (O, C, I, J, K) -> [(dj,c,di)=288, i, j, o]
    w_r = weight.reshape(C_OUT, C_IN, OH, OW, KH, KW)
    w_t = np.ascontiguousarray(
        w_r.transpose(5, 1, 4, 2, 3, 0).reshape(288, OH, OW, C_OUT)
    ).astype(np_mm)

    in_maps = []
    for m in range(N_CORES):
        r0 = m * RPC
        in_maps.append({
            "x": np.ascontiguousarray(x_t[:, r0:r0 + HALO]),
            "w": np.ascontiguousarray(w_t[:, r0:r0 + RPC]),
        })
    return in_maps


def _gather(results, bias):
    out_full = np.empty((B, C_OUT, OH, OW), np.float32)
    for m in range(N_CORES):
        r = results[m]["out"].astype(np.float32)          # (128, 4, 16, 128)
        r = r.reshape(2, C_OUT, RPC, NPAIR, B)            # par,o,i,jh,b
        r = r.transpose(4, 1, 2, 3, 0)                    # b,o,i,jh,par
        out_full[:, :, m * RPC:(m + 1) * RPC, :] = r.reshape(B, C_OUT, RPC, OW)
    out_full += bias[None].astype(np.float32)
    return out_full


def kernel(x, weight, bias, _trace=False):
    from concourse.bass_utils import run_bass_kernel_spmd

    if "nc" not in _prog_cache:
        _prog_cache["nc"] = _build_program()
    nc = _prog_cache["nc"]

    in_maps = _host_prep(np.asarray(x), np.asarray(weight))
    res = run_bass_kernel_spmd(nc, in_maps, core_ids=list(range(N_CORES)),
                               trace=_trace)
    out = _gather(res.results, np.asarray(bias))
    if _trace:
        _prog_cache["last_result"] = res
    return out
